# revision 1
# baseline (speedup 1.0000x reference)
"""Trainium2 Bass kernel for a 3-layer GCN encoder with global max pool.

Strategy (8 NeuronCores, SPMD, 4 launches):
  - Nodes are partitioned graph-wise across cores (graphs g -> core g//64),
    degree-sorted within each core so padded-CSR gather tiles are tight.
  - GCN normalization is factored: out = s * Agg(s * h), s = 1/sqrt(deg),
    Agg includes the self loop via a separate "self" add (no per-edge norm).
  - Matmuls are reordered to keep every gather 64 features wide:
      L1: T1 = s*(X @ W1);  h1 = relu(s*Agg(T1) + b1)
      L2: T2 = s*h1;        h2 = relu((s*Agg(T2)) @ W2 + b2)
      L3: T3 = s*(h2 @ W3); out = s*Agg(T3) + b3
  - Launch 1 builds the T1 table shard per core; the host concatenates shards
    (the "AllGather") and feeds the full table to the next launch.  Launches
    2/3/4 aggregate with dma_gather (padded CSR, one 256B descriptor per edge
    row; int16 indices so the 51K-row table is gathered as a low half + high
    half) and a single strided DVE tensor_reduce per 128-node tile.
  - Global max pool (launch 4): per-core padded node lists per graph (2 SBUF
    partitions per graph), dma_gather + tensor_reduce(max) + PE transpose +
    pairwise max.
"""

import numpy as np

N_NODES = 50000
N_EDGES = 600000
IN_DIM = 128
HID = 64
N_GRAPHS = 512
C = 8           # cores
P = 128         # partitions
GPC = N_GRAPHS // C
SPLIT = 32768   # int16 index range per dma_gather call


def _pack_idx16(flat):
    """[num] int array -> wrapped [128, num//16] int16 (16-wrapped, 8x repl)."""
    num = flat.shape[0]
    assert num % 16 == 0
    arr = flat.reshape(num // 16, 16).T.astype(np.int16)   # [16, num//16]
    return np.tile(arr, (8, 1))                            # [128, num//16]


# --------------------------------------------------------------------------
# Host-side preprocessing: sharding, permutations, padded CSR index arrays.
# --------------------------------------------------------------------------

def _host_prep(data, edge_index, batch):
    N = data.shape[0]
    src = np.asarray(edge_index[0], dtype=np.int64)
    dst = np.asarray(edge_index[1], dtype=np.int64)
    batch = np.asarray(batch, dtype=np.int64)

    indeg = np.bincount(dst, minlength=N)
    deg = (indeg + 1).astype(np.float32)
    s = (1.0 / np.sqrt(deg)).astype(np.float32)

    core_of_node = batch // GPC

    # pass 1: degree sort to fix an initial table layout, from which each
    # node's low/high-half in-edge counts (kA/kB) are estimated; pass 2
    # re-sorts by (kA, kB) so padded-CSR tiles are tight on BOTH gather calls.
    perms = []
    for c in range(C):
        nodes_c = np.nonzero(core_of_node == c)[0]
        order = np.argsort(indeg[nodes_c], kind="stable")
        perms.append(nodes_c[order])
    maxlen = max(len(p) for p in perms)
    T0 = -(-maxlen // P)
    Npc0 = T0 * P
    tab0 = np.full(N, -1, np.int64)
    for c in range(C):
        tab0[perms[c]] = c * Npc0 + np.arange(len(perms[c]))
    low0 = (tab0[src] + 1) < SPLIT
    kA = np.bincount(dst[low0], minlength=N)
    kB = indeg - kA
    perms = []
    for c in range(C):
        nodes_c = np.nonzero(core_of_node == c)[0]
        order = np.lexsort((kB[nodes_c], kA[nodes_c]))
        perms.append(nodes_c[order])
    T = -(-maxlen // P)
    Npc = T * P
    NT = C * Npc
    ZB = (NT + 1) - SPLIT  # high-half index of the trailing zero row

    tab_of_node = np.full(N, -1, np.int64)
    node_of_row = np.full(NT, -1, np.int64)
    for c in range(C):
        rows = c * Npc + np.arange(len(perms[c]))
        tab_of_node[perms[c]] = rows
        node_of_row[rows] = perms[c]

    vrow = tab_of_node[dst]
    vcore = vrow // Npc
    vloc = vrow % Npc
    vtile = vloc // P
    vpart = vloc % P
    urow = tab_of_node[src] + 1          # +1: table row of the source node
    lowmask = urow < SPLIT

    # per-call slot counts: DA/DB = max over (core, partition) of per-node
    # low/high in-edge counts, per tile index (uniform across cores for SPMD)
    def csr_side(mask, sub):
        cnt = np.zeros((C, T, P), np.int64)
        np.add.at(cnt, (vcore[mask], vtile[mask], vpart[mask]), 1)
        D_t = cnt.max(axis=2).max(axis=0)
        order = np.lexsort((vpart[mask], vtile[mask], vcore[mask]))
        vc, vt, vp = vcore[mask][order], vtile[mask][order], vpart[mask][order]
        uo = sub[mask][order]
        key = (vc * T + vt) * P + vp
        newgrp = np.concatenate([[True], key[1:] != key[:-1]])
        gsp = np.nonzero(newgrp)[0]
        slot = np.arange(len(key)) - gsp[np.cumsum(newgrp) - 1]
        return D_t, (vc, vt, vp, slot, uo)

    DA_t, edA = csr_side(lowmask, urow)
    DB_t, edB = csr_side(~lowmask, urow - SPLIT)

    # tile groups of G: uniform DAg/DBg within a group so one dma_gather
    # covers the whole group's slots
    G = 4
    ngroups = -(-T // G)
    DAg = [int(DA_t[g * G : min((g + 1) * G, T)].max()) for g in range(ngroups)]
    DBg = [int(DB_t[g * G : min((g + 1) * G, T)].max()) for g in range(ngroups)]
    Gsz = [min((g + 1) * G, T) - g * G for g in range(ngroups)]
    idx_flat_A = [[np.zeros(P * Gsz[g] * DAg[g], np.int64)
                   for g in range(ngroups)] for _ in range(C)]
    idx_flat_B = [[np.full(P * Gsz[g] * DBg[g], ZB, np.int64)
                   for g in range(ngroups)] for _ in range(C)]
    vc, vt, vp, slot, uo = edA
    vg = vt // G
    tloc = vt % G
    DAg_e = np.array(DAg)[vg]
    pos = (tloc * DAg_e + slot) * P + vp
    for c in range(C):
        for g in range(ngroups):
            m = (vc == c) & (vg == g)
            idx_flat_A[c][g][pos[m]] = uo[m]
    vc, vt, vp, slot, uo = edB
    vg = vt // G
    tloc = vt % G
    DBg_e = np.array(DBg)[vg]
    pos = (tloc * DBg_e + slot) * P + vp
    for c in range(C):
        for g in range(ngroups):
            m = (vc == c) & (vg == g)
            idx_flat_B[c][g][pos[m]] = uo[m]

    # pooling CSR: graph local slot l -> partitions 2l, 2l+1 (alternating)
    loc_of_node = tab_of_node % Npc
    gl = batch % GPC
    okey = core_of_node * (GPC * Npc) + gl * Npc + loc_of_node
    oorder = np.argsort(okey)
    oc = core_of_node[oorder]
    ogl = gl[oorder]
    oloc = loc_of_node[oorder]
    gkey = oc * GPC + ogl
    gnew = np.concatenate([[True], gkey[1:] != gkey[:-1]])
    gsp = np.nonzero(gnew)[0]
    gslot = np.arange(len(gkey)) - gsp[np.cumsum(gnew) - 1]
    ppart = (2 * ogl + (gslot % 2)).astype(np.int64)
    pslot = gslot // 2
    Dp = int(pslot.max()) + 1
    pool_flat = np.zeros((C, P * Dp), np.int64)   # pad -> row 0 (-inf row)
    pool_flat[oc, pslot * P + ppart] = oloc + 1   # +1: out3_local row shift

    # concatenate packed gather-index blocks per core
    idx16 = []
    for c in range(C):
        cols = []
        col = 0
        groups = []
        for g in range(ngroups):
            nA = Gsz[g] * DAg[g]
            nB = Gsz[g] * DBg[g]
            cA = col
            if nA:
                cols.append(_pack_idx16(idx_flat_A[c][g]))
                col += 8 * nA
            cB = col
            if nB:
                cols.append(_pack_idx16(idx_flat_B[c][g]))
                col += 8 * nB
            groups.append((cA, DAg[g], cB, DBg[g], g * G, Gsz[g]))
        idx16.append(np.concatenate(cols, axis=1) if cols else
                     np.zeros((P, 0), np.int16))
        COLS16 = col
    idx16 = np.stack(idx16)     # [C, 128, COLS16]
    pool16 = np.stack([_pack_idx16(pool_flat[c]) for c in range(C)])

    dinvT = np.zeros((C, P, T), np.float32)
    valid = node_of_row >= 0
    rr = np.arange(NT)[valid]
    dinvT[rr // Npc, rr % P, (rr % Npc) // P] = s[node_of_row[valid]]

    XT = np.zeros((C, IN_DIM, Npc), np.float32)
    X = np.asarray(data, dtype=np.float32)
    for c in range(C):
        XT[c, :, : len(perms[c])] = X[perms[c]].T

    meta = dict(T=T, Npc=Npc, COLS16=COLS16, groups=groups,
                Dp=Dp, GDMAX=max(Gsz[g] * (DAg[g] + DBg[g])
                                 for g in range(ngroups)))
    return dict(idx16=idx16, pool16=pool16, dinvT=dinvT, XT=XT, meta=meta)


# --------------------------------------------------------------------------
# Bass programs (4 launches)
# --------------------------------------------------------------------------

def _mk_bass():
    import concourse.bacc as bacc
    return bacc.Bacc(None)


def _gather_fold(nc, tc, meta, TBL_d, IDX_s, agg_strip, msgp, ztp):
    import concourse.mybir as mybir
    f32 = mybir.dt.float32
    Alu = mybir.AluOpType
    Axis = mybir.AxisListType
    GDMAX = meta["GDMAX"]
    NTAB = C * meta["Npc"] + 2
    for (cA, DA, cB, DB, t0, gsz) in meta["groups"]:
        nA = gsz * DA
        nB = gsz * DB
        if nA + nB == 0:
            for t in range(t0, t0 + gsz):
                nc.vector.memset(agg_strip[:, t * HID : (t + 1) * HID], 0.0)
            continue
        msg = msgp.tile([P, GDMAX * HID], f32, tag="msg")
        if nA:
            nc.gpsimd.dma_gather(
                out_ap=msg[:, : nA * HID].rearrange("p (d f) -> p d f", f=HID),
                in_ap=TBL_d[0 : min(SPLIT, NTAB), :],
                idxs_ap=IDX_s[:, cA : cA + 8 * nA],
                num_idxs=P * nA,
                num_idxs_reg=P * nA,
                elem_size=HID,
                single_packet=False,
            )
        if nB:
            nc.gpsimd.dma_gather(
                out_ap=msg[:, nA * HID : (nA + nB) * HID]
                    .rearrange("p (d f) -> p d f", f=HID),
                in_ap=TBL_d[SPLIT:NTAB, :],
                idxs_ap=IDX_s[:, cB : cB + 8 * nB],
                num_idxs=P * nB,
                num_idxs_reg=P * nB,
                elem_size=HID,
                single_packet=False,
            )
        for i in range(gsz):
            t = t0 + i
            sl = agg_strip[:, t * HID : (t + 1) * HID]
            if DA:
                nc.vector.tensor_reduce(
                    out=sl,
                    in_=msg[:, i * DA * HID : (i + 1) * DA * HID]
                        .rearrange("p (d f) -> p f d", f=HID),
                    axis=Axis.X,
                    op=Alu.add,
                )
            if DB:
                bofs = nA + i * DB
                bap = msg[:, bofs * HID : (bofs + DB) * HID] \
                    .rearrange("p (d f) -> p f d", f=HID)
                if DA:
                    tmp = ztp.tile([P, HID], f32, tag="btmp")
                    nc.vector.tensor_reduce(out=tmp[:], in_=bap,
                                            axis=Axis.X, op=Alu.add)
                    nc.vector.tensor_tensor(out=sl, in0=sl, in1=tmp[:],
                                            op=Alu.add)
                else:
                    nc.vector.tensor_reduce(out=sl, in_=bap,
                                            axis=Axis.X, op=Alu.add)
            if not DA and not DB:
                nc.vector.memset(sl, 0.0)


def _prog_tables(meta):
    """Launch 1: T1 shard = s * (X @ W1) for this core's nodes."""
    import concourse.mybir as mybir
    import concourse.tile as tile

    T, Npc = meta["T"], meta["Npc"]
    f32 = mybir.dt.float32
    Alu = mybir.AluOpType
    nc = _mk_bass()

    XT_d = nc.dram_tensor("XT", [IN_DIM, Npc], f32, kind="ExternalInput")
    W1_d = nc.dram_tensor("W1", [IN_DIM, HID], f32, kind="ExternalInput")
    DINV_d = nc.dram_tensor("DINV", [P, T], f32, kind="ExternalInput")
    T1S_d = nc.dram_tensor("T1S", [Npc, HID], f32, kind="ExternalOutput")

    with tile.TileContext(nc, num_cores=C) as tc:
        with (
            tc.tile_pool(name="const", bufs=1) as const,
            tc.tile_pool(name="zt", bufs=3) as ztp,
            tc.tile_pool(name="psum", bufs=3, space="PSUM") as psp,
        ):
            XT_s = const.tile([IN_DIM, Npc], f32)
            nc.sync.dma_start(XT_s[:], XT_d[:])
            W1_s = const.tile([IN_DIM, HID], f32)
            nc.sync.dma_start(W1_s[:], W1_d[:])
            DINV_s = const.tile([P, T], f32)
            nc.sync.dma_start(DINV_s[:], DINV_d[:])
            W1_pe = const.tile([IN_DIM, HID], f32)
            nc.vector.tensor_copy(W1_pe[:], W1_s[:])
            strip = const.tile([P, T * HID], f32)
            for t in range(T):
                xt_t = ztp.tile([IN_DIM, P], f32, tag="xt")
                nc.vector.tensor_copy(xt_t[:], XT_s[:, t * P : (t + 1) * P])
                ps = psp.tile([P, HID], f32, tag="ps")
                nc.tensor.matmul(ps[:], lhsT=xt_t[:], rhs=W1_pe[:],
                                 start=True, stop=True)
                nc.vector.tensor_copy(strip[:, t * HID : (t + 1) * HID], ps[:])
            nc.vector.tensor_tensor(
                out=strip[:].rearrange("p (t f) -> p t f", f=HID),
                in0=strip[:].rearrange("p (t f) -> p t f", f=HID),
                in1=DINV_s[:, :, None].to_broadcast([P, T, HID]),
                op=Alu.mult,
            )
            for t in range(T):
                nc.sync.dma_start(T1S_d[t * P : (t + 1) * P, :],
                                  strip[:, t * HID : (t + 1) * HID])
    nc.compile()
    return nc


def _prog_layer(meta, layer):
    """Launches 2/3: aggregate TBL -> next table shard.

    layer=1: out = s * relu(s*Agg(T1) + b1)                  (T2 shard)
    layer=2: out = s * ((relu((s*Agg(T2)) @ W2 + b2)) @ W3)  (T3 shard)
    """
    import concourse.mybir as mybir
    import concourse.tile as tile
    from concourse.masks import make_identity

    T, Npc = meta["T"], meta["Npc"]
    COLS16 = meta["COLS16"]
    NT = C * Npc
    NTAB = NT + 2
    f32 = mybir.dt.float32
    i16 = mybir.dt.int16
    Alu = mybir.AluOpType
    nc = _mk_bass()

    TBL_d = nc.dram_tensor("TBL", [NTAB, HID], f32, kind="ExternalInput")
    SELF_d = nc.dram_tensor("SELF", [P, T * HID], f32, kind="ExternalInput")
    DINV_d = nc.dram_tensor("DINV", [P, T], f32, kind="ExternalInput")
    IDX_d = nc.dram_tensor("IDX16", [P, COLS16], i16, kind="ExternalInput")
    OUTS_d = nc.dram_tensor("OUTS", [Npc, HID], f32, kind="ExternalOutput")
    if layer == 1:
        B_d = nc.dram_tensor("B", [P, HID], f32, kind="ExternalInput")
    else:
        B_d = nc.dram_tensor("B", [P, 2 * HID], f32, kind="ExternalInput")
        W2_d = nc.dram_tensor("W2", [HID, 2 * HID], f32, kind="ExternalInput")
        W3_d = nc.dram_tensor("W3", [2 * HID, HID], f32, kind="ExternalInput")

    with tile.TileContext(nc, num_cores=C) as tc:
        with (
            tc.tile_pool(name="const", bufs=1) as const,
            tc.tile_pool(name="msg", bufs=3) as msgp,
            tc.tile_pool(name="zt", bufs=3) as ztp,
            tc.tile_pool(name="psum", bufs=3, space="PSUM") as psp,
        ):
            SELF_s = const.tile([P, T * HID], f32)
            nc.sync.dma_start(SELF_s[:], SELF_d[:])
            DINV_s = const.tile([P, T], f32)
            nc.sync.dma_start(DINV_s[:], DINV_d[:])
            IDX_s = const.tile([P, COLS16], i16)
            nc.sync.dma_start(IDX_s[:], IDX_d[:])
            B_s = const.tile([P, HID if layer == 1 else 2 * HID], f32)
            nc.sync.dma_start(B_s[:], B_d[:])
            if layer == 2:
                W2_s = const.tile([HID, 2 * HID], f32)
                nc.sync.dma_start(W2_s[:], W2_d[:])
                W3_s = const.tile([2 * HID, HID], f32)
                nc.sync.dma_start(W3_s[:], W3_d[:])
                W2_pe = const.tile([HID, 2 * HID], f32)
                nc.vector.tensor_copy(W2_pe[:], W2_s[:])
                W3_pe = const.tile([2 * HID, HID], f32)
                nc.vector.tensor_copy(W3_pe[:], W3_s[:])
                ident = const.tile([P, P], f32)
                make_identity(nc, ident[:])
                ident_pe = const.tile([P, P], f32)
                nc.vector.tensor_copy(ident_pe[:], ident[:])
                h2_strip = const.tile([P, T * 2 * HID], f32)
            agg_strip = const.tile([P, T * HID], f32)
            out_strip = const.tile([P, T * HID], f32)

            def strip3(strip, F):
                return strip[:].rearrange("p (t f) -> p t f", f=F)

            def bcast_dinv(F):
                return DINV_s[:, :, None].to_broadcast([P, T, F])

            def bcast_bias(F):
                return B_s[:, None, :].to_broadcast([P, T, F])

            _gather_fold(nc, tc, meta, TBL_d, IDX_s, agg_strip, msgp, ztp)
            nc.vector.tensor_tensor(out=agg_strip[:], in0=agg_strip[:],
                                    in1=SELF_s[:], op=Alu.add)
            nc.vector.tensor_tensor(
                out=strip3(agg_strip, HID), in0=strip3(agg_strip, HID),
                in1=bcast_dinv(HID), op=Alu.mult,
            )
            if layer == 1:
                nc.vector.tensor_tensor(
                    out=strip3(agg_strip, HID), in0=strip3(agg_strip, HID),
                    in1=bcast_bias(HID), op=Alu.add,
                )
                nc.vector.tensor_scalar(out=out_strip[:], in0=agg_strip[:],
                                        scalar1=0.0, scalar2=None, op0=Alu.max)
                nc.vector.tensor_tensor(
                    out=strip3(out_strip, HID), in0=strip3(out_strip, HID),
                    in1=bcast_dinv(HID), op=Alu.mult,
                )
            else:
                for t in range(T):
                    psT = psp.tile([HID, P], f32, tag="psT")
                    nc.tensor.transpose(
                        psT[:], agg_strip[:, t * HID : (t + 1) * HID],
                        ident_pe[:],
                    )
                    zT = ztp.tile([HID, P], f32, tag="zT")
                    nc.vector.tensor_copy(zT[:], psT[:])
                    ps2 = psp.tile([P, 2 * HID], f32, tag="ps")
                    nc.tensor.matmul(ps2[:], lhsT=zT[:], rhs=W2_pe[:],
                                     start=True, stop=True)
                    nc.vector.tensor_copy(
                        h2_strip[:, t * 2 * HID : (t + 1) * 2 * HID], ps2[:]
                    )
                nc.vector.tensor_tensor(
                    out=strip3(h2_strip, 2 * HID),
                    in0=strip3(h2_strip, 2 * HID),
                    in1=bcast_bias(2 * HID), op=Alu.add,
                )
                nc.vector.tensor_scalar(out=h2_strip[:], in0=h2_strip[:],
                                        scalar1=0.0, scalar2=None, op0=Alu.max)
                for t in range(T):
                    psT2 = psp.tile([P, P], f32, tag="psT")
                    nc.tensor.transpose(
                        psT2[:], h2_strip[:, t * 2 * HID : (t + 1) * 2 * HID],
                        ident_pe[:],
                    )
                    hT = ztp.tile([P, P], f32, tag="hT")
                    nc.vector.tensor_copy(hT[:], psT2[:])
                    ps3 = psp.tile([P, HID], f32, tag="ps")
                    nc.tensor.matmul(ps3[:], lhsT=hT[:], rhs=W3_pe[:],
                                     start=True, stop=True)
                    nc.vector.tensor_copy(
                        out_strip[:, t * HID : (t + 1) * HID], ps3[:]
                    )
                nc.vector.tensor_tensor(
                    out=strip3(out_strip, HID), in0=strip3(out_strip, HID),
                    in1=bcast_dinv(HID), op=Alu.mult,
                )
            for t in range(T):
                nc.sync.dma_start(OUTS_d[t * P : (t + 1) * P, :],
                                  out_strip[:, t * HID : (t + 1) * HID])
    nc.compile()
    return nc


def _prog_final(meta):
    """Launch 4: layer-3 aggregation + bias, then global max pool."""
    import concourse.mybir as mybir
    import concourse.tile as tile
    from concourse.masks import make_identity

    T, Npc, Dp = meta["T"], meta["Npc"], meta["Dp"]
    COLS16 = meta["COLS16"]
    NT = C * Npc
    NTAB = NT + 2
    f32 = mybir.dt.float32
    i16 = mybir.dt.int16
    Alu = mybir.AluOpType
    Axis = mybir.AxisListType
    nc = _mk_bass()

    TBL_d = nc.dram_tensor("TBL", [NTAB, HID], f32, kind="ExternalInput")
    SELF_d = nc.dram_tensor("SELF", [P, T * HID], f32, kind="ExternalInput")
    DINV_d = nc.dram_tensor("DINV", [P, T], f32, kind="ExternalInput")
    IDX_d = nc.dram_tensor("IDX16", [P, COLS16], i16, kind="ExternalInput")
    PIDX_d = nc.dram_tensor("PIDX", [P, 8 * Dp], i16, kind="ExternalInput")
    B_d = nc.dram_tensor("B", [P, HID], f32, kind="ExternalInput")
    OUT_d = nc.dram_tensor("OUT", [HID, GPC], f32, kind="ExternalOutput")

    out3_local = nc.dram_tensor("out3_local", [Npc + 1, HID], f32)

    with tile.TileContext(nc, num_cores=C) as tc:
        with (
            tc.tile_pool(name="const", bufs=1) as const,
            tc.tile_pool(name="msg", bufs=3) as msgp,
            tc.tile_pool(name="zt", bufs=3) as ztp,
            tc.tile_pool(name="psum", bufs=3, space="PSUM") as psp,
        ):
            SELF_s = const.tile([P, T * HID], f32)
            nc.sync.dma_start(SELF_s[:], SELF_d[:])
            DINV_s = const.tile([P, T], f32)
            nc.sync.dma_start(DINV_s[:], DINV_d[:])
            IDX_s = const.tile([P, COLS16], i16)
            nc.sync.dma_start(IDX_s[:], IDX_d[:])
            PIDX_s = const.tile([P, 8 * Dp], i16)
            nc.sync.dma_start(PIDX_s[:], PIDX_d[:])
            B_s = const.tile([P, HID], f32)
            nc.sync.dma_start(B_s[:], B_d[:])
            ident = const.tile([P, P], f32)
            make_identity(nc, ident[:])
            ident_pe = const.tile([P, P], f32)
            nc.vector.tensor_copy(ident_pe[:], ident[:])
            nirow = const.tile([1, HID], f32)
            nc.vector.memset(nirow[:], float("-inf"))
            nc.sync.dma_start(out3_local[0:1, :], nirow[:])
            agg_strip = const.tile([P, T * HID], f32)

            def strip3(strip, F):
                return strip[:].rearrange("p (t f) -> p t f", f=F)

            _gather_fold(nc, tc, meta, TBL_d, IDX_s, agg_strip, msgp, ztp)
            nc.vector.tensor_tensor(out=agg_strip[:], in0=agg_strip[:],
                                    in1=SELF_s[:], op=Alu.add)
            nc.vector.tensor_tensor(
                out=strip3(agg_strip, HID), in0=strip3(agg_strip, HID),
                in1=DINV_s[:, :, None].to_broadcast([P, T, HID]), op=Alu.mult,
            )
            nc.vector.tensor_tensor(
                out=strip3(agg_strip, HID), in0=strip3(agg_strip, HID),
                in1=B_s[:, None, :].to_broadcast([P, T, HID]), op=Alu.add,
            )
            for t in range(T):
                nc.sync.dma_start(out3_local[1 + t * P : 1 + (t + 1) * P, :],
                                  agg_strip[:, t * HID : (t + 1) * HID])

            pmsg = msgp.tile([P, Dp * HID], f32, tag="pmsg")
            nc.gpsimd.dma_gather(
                out_ap=pmsg[:].rearrange("p (d f) -> p d f", f=HID),
                in_ap=out3_local[:],
                idxs_ap=PIDX_s[:],
                num_idxs=P * Dp,
                num_idxs_reg=P * Dp,
                elem_size=HID,
                single_packet=False,
            )
            poolA = ztp.tile([P, HID], f32, tag="poolA")
            nc.vector.tensor_reduce(
                out=poolA[:],
                in_=pmsg[:].rearrange("p (d f) -> p f d", f=HID),
                axis=Axis.X,
                op=Alu.max,
            )
            psP = psp.tile([HID, P], f32, tag="psT")
            nc.tensor.transpose(psP[:], poolA[:], ident_pe[:])
            poolT = ztp.tile([HID, P], f32, tag="poolT")
            nc.vector.tensor_copy(poolT[:], psP[:])
            outsb = ztp.tile([HID, GPC], f32, tag="outsb")
            pt = poolT[:].rearrange("p (g two) -> p g two", two=2)
            nc.vector.tensor_tensor(out=outsb[:], in0=pt[:, :, 0],
                                    in1=pt[:, :, 1], op=Alu.max)
            nc.sync.dma_start(OUT_d[:], outsb[:])
    nc.compile()
    return nc


# --------------------------------------------------------------------------
# Entry point
# --------------------------------------------------------------------------

_RUN_KWARGS = {}
_EXEC_NS = []    # per-launch HW exec times when tracing enabled
_PROFILE = False


def _concat_table(shards, Npc):
    """Host 'AllGather': [C][Npc, HID] -> [NT+2, HID] with zero guard rows."""
    NT = C * Npc
    tab = np.zeros((NT + 2, HID), np.float32)
    for c in range(C):
        tab[1 + c * Npc : 1 + (c + 1) * Npc] = shards[c]
    return tab


def _strip_of(shard, T):
    """[Npc, HID] -> [128, T*HID] strip layout."""
    return np.ascontiguousarray(
        shard.reshape(T, P, HID).transpose(1, 0, 2).reshape(P, T * HID))


def kernel(data, edge_index, batch, W1, b1, W2, b2, W3, b3):
    from concourse.bass_utils import run_bass_kernel_spmd

    data = np.asarray(data, dtype=np.float32)
    edge_index = np.asarray(edge_index, dtype=np.int32)
    batch_np = np.asarray(batch, dtype=np.int32)
    W1 = np.asarray(W1, dtype=np.float32)
    b1 = np.asarray(b1, dtype=np.float32)
    W2 = np.asarray(W2, dtype=np.float32)
    b2 = np.asarray(b2, dtype=np.float32)
    W3 = np.asarray(W3, dtype=np.float32)
    b3 = np.asarray(b3, dtype=np.float32)

    prep = _host_prep(data, edge_index, batch_np)
    meta = prep["meta"]
    T, Npc = meta["T"], meta["Npc"]

    B1 = np.broadcast_to(b1, (P, HID)).copy()
    B2 = np.broadcast_to(b2, (P, 2 * HID)).copy()
    B3 = np.broadcast_to(b3, (P, HID)).copy()
    cores = list(range(C))
    del _EXEC_NS[:]

    def run(nc, in_maps):
        if _PROFILE:
            from concourse.timeline_sim import TimelineSim
            _EXEC_NS.append(TimelineSim(nc, require_finite=False).simulate())
        res = run_bass_kernel_spmd(nc, in_maps, cores, **_RUN_KWARGS)
        if res.exec_time_ns is not None:
            _EXEC_NS.append(res.exec_time_ns)
        return res.results

    # ---- launch 1: T1 tables ----
    nc1 = _prog_tables(meta)
    r1 = run(nc1, [{"XT": np.ascontiguousarray(prep["XT"][c]),
                    "W1": W1,
                    "DINV": np.ascontiguousarray(prep["dinvT"][c])}
                   for c in range(C)])
    t1_shards = [np.asarray(r1[c]["T1S"]) for c in range(C)]
    t1f = _concat_table(t1_shards, Npc)

    # ---- launch 2: layer 1 -> T2 ----
    nc2 = _prog_layer(meta, 1)
    r2 = run(nc2, [{"TBL": t1f,
                    "SELF": _strip_of(t1_shards[c], T),
                    "DINV": np.ascontiguousarray(prep["dinvT"][c]),
                    "IDX16": np.ascontiguousarray(prep["idx16"][c]),
                    "B": B1}
                   for c in range(C)])
    t2_shards = [np.asarray(r2[c]["OUTS"]) for c in range(C)]
    t2f = _concat_table(t2_shards, Npc)

    # ---- launch 3: layer 2 -> T3 ----
    nc3 = _prog_layer(meta, 2)
    r3 = run(nc3, [{"TBL": t2f,
                    "SELF": _strip_of(t2_shards[c], T),
                    "DINV": np.ascontiguousarray(prep["dinvT"][c]),
                    "IDX16": np.ascontiguousarray(prep["idx16"][c]),
                    "B": B2, "W2": W2, "W3": W3}
                   for c in range(C)])
    t3_shards = [np.asarray(r3[c]["OUTS"]) for c in range(C)]
    t3f = _concat_table(t3_shards, Npc)

    # ---- launch 4: layer 3 + pool ----
    nc4 = _prog_final(meta)
    r4 = run(nc4, [{"TBL": t3f,
                    "SELF": _strip_of(t3_shards[c], T),
                    "DINV": np.ascontiguousarray(prep["dinvT"][c]),
                    "IDX16": np.ascontiguousarray(prep["idx16"][c]),
                    "PIDX": np.ascontiguousarray(prep["pool16"][c]),
                    "B": B3}
                   for c in range(C)])
    out = np.concatenate(
        [np.asarray(r4[c]["OUT"]).T for c in range(C)], axis=0
    )
    return out.astype(np.float32)



# revision 7
# speedup vs baseline: 1.5850x; 1.5850x over previous
"""Trainium2 Bass kernel for a 3-layer GCN encoder with global max pool.

Strategy (8 NeuronCores, SPMD, 4 launches):
  - Nodes are partitioned graph-wise across cores (graphs g -> core g//64),
    degree-sorted within each core so padded-CSR tiles are nearly exact
    (~1.10x padding).
  - GCN normalization is factored: out = s * Agg(s * h), s = 1/sqrt(deg);
    the self loop is a separate "self" add of the core's own table strip.
  - The inter-layer node table is fp16 with TWO nodes per 256-byte row
    ("pair table", 25601 rows), so one int16 index space covers all 51200
    table rows with no low/high split.  Each padded-CSR slot gathers one
    256B pair row; a static uint8 mask selects the needed half:
    ACT copies the hi half, DVE copy_predicated overwrites with the lo
    half where the mask is set (pad slots: row 0 + mask 0 -> zeros).
  - Slots are laid out tile-major with EXACT per-tile depth D_t (max node
    degree within the 128-node tile across all 8 cores); gathers are
    chunked (~6 calls) for DMA/DVE pipelining and reduces run per
    uniform-D run of tiles.
  - Matmuls are reordered to keep every gather 64 features wide:
      L1: T1 = s*(X @ W1);  L2: h1 = relu(s*Agg(T1) + b1), T2 = s*h1
      L3: u = s*Agg(T2); h2 = relu(u @ W2 + b2); T3 = s*(h2 @ W3)
      L4: out3 = s*Agg(T3) + b3; per-graph max pool.
  - The host concatenates per-core strip outputs into the next pair table
    between launches (the "AllGather" costs no device time).
"""

import numpy as np

N_NODES = 50000
N_EDGES = 600000
IN_DIM = 128
HID = 64
N_GRAPHS = 512
C = 8           # cores
P = 128         # partitions
GPC = N_GRAPHS // C


def _pack_idx16(flat):
    """[num] int array -> wrapped [128, num//16] int16 (16-wrapped, 8x repl)."""
    num = flat.shape[0]
    assert num % 16 == 0
    arr = flat.reshape(num // 16, 16).T.astype(np.int16)   # [16, num//16]
    return np.tile(arr, (8, 1))                            # [128, num//16]


# --------------------------------------------------------------------------
# Host-side preprocessing: sharding, permutations, padded CSR index arrays.
# --------------------------------------------------------------------------

def _host_prep(data, edge_index, batch):
    N = data.shape[0]
    src = np.asarray(edge_index[0], dtype=np.int64)
    dst = np.asarray(edge_index[1], dtype=np.int64)
    batch = np.asarray(batch, dtype=np.int64)

    indeg = np.bincount(dst, minlength=N)
    s = (1.0 / np.sqrt((indeg + 1).astype(np.float32))).astype(np.float32)
    core_of_node = batch // GPC

    # degree-sorted rank within each core; rank r -> tile r//P, partition r%P
    perms = []
    for c in range(C):
        nodes_c = np.nonzero(core_of_node == c)[0]
        order = np.argsort(indeg[nodes_c], kind="stable")
        perms.append(nodes_c[order])
    maxlen = max(len(p) for p in perms)
    T = -(-maxlen // P)
    Npc = T * P
    NT = C * Npc
    NPAIR = NT // 2                     # pair rows (table row 0 is the guard)

    tab_of_node = np.full(N, -1, np.int64)
    for c in range(C):
        tab_of_node[perms[c]] = c * Npc + np.arange(len(perms[c]))

    # exact per-tile depth D_t = max indeg among nodes of tile t (all cores)
    deg_of_rank = np.zeros((C, Npc), np.int64)
    for c in range(C):
        deg_of_rank[c, : len(perms[c])] = indeg[perms[c]]
    D_t = deg_of_rank.reshape(C, T, P).max(axis=2).max(axis=0)   # [T]
    slot_off = np.concatenate([[0], np.cumsum(D_t)])             # [T+1]
    S = int(slot_off[-1])

    # edge -> (core, tile, part, slot-within-node)
    vrank = tab_of_node[dst]
    vcore = vrank // Npc
    vloc = vrank % Npc
    vtile = vloc // P
    vpart = vloc % P
    order = np.lexsort((vpart, vtile, vcore))
    ec, et, ep = vcore[order], vtile[order], vpart[order]
    es = src[order]
    key = (ec * T + et) * P + ep
    newgrp = np.concatenate([[True], key[1:] != key[:-1]])
    gsp = np.nonzero(newgrp)[0]
    eslot = np.arange(len(key)) - gsp[np.cumsum(newgrp) - 1]

    # per-core idx (pair row) + lo-half mask arrays, tile-major slot layout
    srow = tab_of_node[es]              # global node rank of the source
    pos = (slot_off[et] + eslot) * P + ep
    idx_flat = np.zeros((C, P * S), np.int64)       # pad -> pair row 0
    mlo_flat = np.zeros((C, P * S), np.uint8)       # pad -> keep hi (zeros)
    for c in range(C):
        m = ec == c
        idx_flat[c][pos[m]] = 1 + srow[m] // 2
        mlo_flat[c][pos[m]] = (srow[m] % 2 == 0).astype(np.uint8)

    # chunks of whole tiles (~balanced slot counts) for gather pipelining
    NCH = 6
    target = S / NCH
    chunks = []          # (t0, t1, slot0, nslots)
    t0 = 0
    for i in range(NCH):
        t1 = T if i == NCH - 1 else int(
            np.searchsorted(slot_off, (i + 1) * target, side="left"))
        t1 = min(max(t1, t0 + 1), T)
        if t0 >= T:
            break
        chunks.append((t0, t1, int(slot_off[t0]),
                       int(slot_off[t1] - slot_off[t0])))
        t0 = t1
    # uniform-D runs within each chunk: (t0, ntiles, D, slot0)
    runs = []
    for (ct0, ct1, cs0, _) in chunks:
        rr = []
        t = ct0
        while t < ct1:
            t2 = t
            while t2 < ct1 and D_t[t2] == D_t[t]:
                t2 += 1
            rr.append((t, t2 - t, int(D_t[t]), int(slot_off[t])))
            t = t2
        runs.append(rr)

    # packed int16 gather indices, chunk-major
    idx16 = []
    for c in range(C):
        cols = []
        for (ct0, ct1, cs0, cn) in chunks:
            if cn:
                cols.append(_pack_idx16(
                    idx_flat[c][cs0 * P: (cs0 + cn) * P]))
        idx16.append(np.concatenate(cols, axis=1) if cols
                     else np.zeros((P, 0), np.int16))
    idx16 = np.stack(idx16)                          # [C, 128, 8*S]
    COLS16 = idx16.shape[2]
    mlo = np.stack([mlo_flat[c].reshape(S, P).T for c in range(C)])  # [C,P,S]

    # dinv per (core, partition, tile)
    dinvT = np.zeros((C, P, T), np.float16)
    for c in range(C):
        n = len(perms[c])
        dv = np.zeros(Npc, np.float32)
        dv[:n] = s[perms[c]]
        dinvT[c] = dv.reshape(T, P).T.astype(np.float16)

    # X^T shard per core (rank-ordered columns), fp16
    XT = np.zeros((C, IN_DIM, Npc), np.float16)
    X = np.asarray(data, dtype=np.float32)
    for c in range(C):
        XT[c, :, : len(perms[c])] = X[perms[c]].T.astype(np.float16)

    # pooling CSR: graph-local g -> partitions 2g, 2g+1 (alternating slots);
    # out3_local row of node (c,t,p) is 1 + p*T + t (row 0 = -inf guard)
    gl = batch % GPC
    loc = tab_of_node % Npc
    tt = loc // P
    pp = loc % P
    okey = core_of_node * (GPC * Npc) + gl * Npc + loc
    oorder = np.argsort(okey)
    oc = core_of_node[oorder]
    ogl = gl[oorder]
    orow = 1 + pp[oorder] * T + tt[oorder]
    gkey = oc * GPC + ogl
    gnew = np.concatenate([[True], gkey[1:] != gkey[:-1]])
    gsp2 = np.nonzero(gnew)[0]
    gslot = np.arange(len(gkey)) - gsp2[np.cumsum(gnew) - 1]
    ppart = 2 * ogl + (gslot % 2)
    pslot = gslot // 2
    Dp = int(pslot.max()) + 1
    pool_flat = np.zeros((C, P * Dp), np.int64)      # pad -> row 0 (-inf)
    pool_flat[oc, pslot * P + ppart] = orow
    pool16 = np.stack([_pack_idx16(pool_flat[c]) for c in range(C)])

    meta = dict(T=T, Npc=Npc, S=S, NPAIR=NPAIR, COLS16=COLS16,
                chunks=chunks, runs=runs, Dp=Dp)
    return dict(idx16=idx16, mlo=mlo, pool16=pool16, dinvT=dinvT, XT=XT,
                meta=meta)


# --------------------------------------------------------------------------
# Bass programs (4 launches)
# --------------------------------------------------------------------------

def _mk_bass():
    import concourse.bacc as bacc
    return bacc.Bacc(None)


def _gather_select(nc, tc, meta, TBL_d, IDX_s, MLO_s, agg_strip, msgp, selp):
    """Chunked gather from the fp16 pair table + half-select + reduces."""
    import concourse.mybir as mybir
    f16 = mybir.dt.float16
    Alu = mybir.AluOpType
    Axis = mybir.AxisListType
    NPAIR = meta["NPAIR"]
    for ci, (ct0, ct1, cs0, cn) in enumerate(meta["chunks"]):
        if cn == 0:
            for (t0, nt, D, s0) in meta["runs"][ci]:
                nc.vector.memset(
                    agg_strip[:, t0 * HID:(t0 + nt) * HID], 0.0)
            continue
        msg = msgp.tile([P, cn * 2 * HID], f16, tag="msg")
        nc.gpsimd.dma_gather(
            out_ap=msg[:].rearrange("p (d f) -> p d f", f=2 * HID),
            in_ap=TBL_d[0:NPAIR + 1, :],
            idxs_ap=IDX_s[:, 8 * cs0: 8 * (cs0 + cn)],
            num_idxs=P * cn,
            num_idxs_reg=P * cn,
            elem_size=2 * HID,
            single_packet=False,
        )
        sel = selp.tile([P, cn * HID], f16, tag="sel")
        msgv = msg[:].rearrange("p (d f) -> p d f", f=2 * HID)
        selv = sel[:].rearrange("p (d f) -> p d f", f=HID)
        nc.scalar.copy(selv, msgv[:, :, HID:2 * HID])
        nc.vector.copy_predicated(
            selv,
            MLO_s[:, cs0:cs0 + cn][:, :, None].to_broadcast([P, cn, HID]),
            msgv[:, :, 0:HID],
        )
        with nc.allow_low_precision(reason="fp16 GCN aggregation"):
            for (t0, nt, D, s0) in meta["runs"][ci]:
                sl = agg_strip[:, t0 * HID:(t0 + nt) * HID]
                if D == 0:
                    nc.vector.memset(sl, 0.0)
                    continue
                lo = s0 - cs0
                nc.vector.tensor_reduce(
                    out=sl.rearrange("p (t f) -> p t f", f=HID),
                    in_=sel[:, lo * HID:(lo + nt * D) * HID]
                        .rearrange("p (t d f) -> p t f d", f=HID, d=D),
                    axis=Axis.X,
                    op=Alu.add,
                )


def _prog_tables(meta):
    """Launch 1: T1 strip = s * (X @ W1) for this core's nodes."""
    import concourse.mybir as mybir
    import concourse.tile as tile

    T, Npc = meta["T"], meta["Npc"]
    f16 = mybir.dt.float16
    f32 = mybir.dt.float32
    Alu = mybir.AluOpType
    nc = _mk_bass()

    XT_d = nc.dram_tensor("XT", [IN_DIM, Npc], f16, kind="ExternalInput")
    W1_d = nc.dram_tensor("W1", [IN_DIM, HID], f16, kind="ExternalInput")
    DINV_d = nc.dram_tensor("DINV", [P, T], f16, kind="ExternalInput")
    OUTS_d = nc.dram_tensor("OUTS", [P, T * HID], f16, kind="ExternalOutput")

    with tile.TileContext(nc, num_cores=C) as tc:
        with (
            tc.tile_pool(name="const", bufs=1) as const,
            tc.tile_pool(name="psum", bufs=4, space="PSUM") as psp,
        ):
            XT_s = const.tile([IN_DIM, Npc], f16)
            nc.sync.dma_start(XT_s[:], XT_d[:])
            W1_s = const.tile([IN_DIM, HID], f16)
            nc.sync.dma_start(W1_s[:], W1_d[:])
            DINV_s = const.tile([P, T], f16)
            nc.sync.dma_start(DINV_s[:], DINV_d[:])
            strip = const.tile([P, T * HID], f16)
            for t in range(T):
                ps = psp.tile([P, HID], f32, tag="ps")
                nc.tensor.matmul(ps[:], lhsT=XT_s[:, t * P:(t + 1) * P],
                                 rhs=W1_s[:], start=True, stop=True)
                nc.vector.tensor_copy(strip[:, t * HID:(t + 1) * HID], ps[:])
            nc.vector.tensor_tensor(
                out=strip[:].rearrange("p (t f) -> p t f", f=HID),
                in0=strip[:].rearrange("p (t f) -> p t f", f=HID),
                in1=DINV_s[:, :, None].to_broadcast([P, T, HID]),
                op=Alu.mult,
            )
            nc.sync.dma_start(OUTS_d[:], strip[:])
    nc.compile()
    return nc


def _layer_io(nc, meta, layer):
    import concourse.mybir as mybir
    T = meta["T"]
    f16 = mybir.dt.float16
    i16 = mybir.dt.int16
    u8 = mybir.dt.uint8
    NPAIR = meta["NPAIR"]
    d = {}
    d["TBL"] = nc.dram_tensor("TBL", [NPAIR + 1, 2 * HID], f16,
                              kind="ExternalInput")
    d["SELF"] = nc.dram_tensor("SELF", [P, T * HID], f16,
                               kind="ExternalInput")
    d["DINV"] = nc.dram_tensor("DINV", [P, T], f16, kind="ExternalInput")
    d["IDX16"] = nc.dram_tensor("IDX16", [P, meta["COLS16"]], i16,
                                kind="ExternalInput")
    d["MLO"] = nc.dram_tensor("MLO", [P, meta["S"]], u8,
                              kind="ExternalInput")
    d["B"] = nc.dram_tensor("B", [P, HID if layer != 2 else 2 * HID], f16,
                            kind="ExternalInput")
    if layer == 2:
        d["W2"] = nc.dram_tensor("W2", [HID, 2 * HID], f16,
                                 kind="ExternalInput")
        d["W3"] = nc.dram_tensor("W3", [2 * HID, HID], f16,
                                 kind="ExternalInput")
    return d


def _load_layer_consts(nc, const, dr, meta, layer):
    import concourse.mybir as mybir
    T = meta["T"]
    f16 = mybir.dt.float16
    i16 = mybir.dt.int16
    u8 = mybir.dt.uint8
    s = {}
    s["IDX"] = const.tile([P, meta["COLS16"]], i16, name="IDXs")
    nc.sync.dma_start(s["IDX"][:], dr["IDX16"][:])
    s["MLO"] = const.tile([P, meta["S"]], u8, name="MLOs")
    nc.sync.dma_start(s["MLO"][:], dr["MLO"][:])
    s["SELF"] = const.tile([P, T * HID], f16, name="SELFs")
    nc.sync.dma_start(s["SELF"][:], dr["SELF"][:])
    s["DINV"] = const.tile([P, T], f16, name="DINVs")
    nc.sync.dma_start(s["DINV"][:], dr["DINV"][:])
    s["B"] = const.tile([P, HID if layer != 2 else 2 * HID], f16, name="Bs")
    nc.sync.dma_start(s["B"][:], dr["B"][:])
    if layer == 2:
        s["W2"] = const.tile([HID, 2 * HID], f16, name="W2s")
        nc.sync.dma_start(s["W2"][:], dr["W2"][:])
        s["W3"] = const.tile([2 * HID, HID], f16, name="W3s")
        nc.sync.dma_start(s["W3"][:], dr["W3"][:])
    return s


def _prog_layer(meta, layer):
    """Launch 2 (layer=1): T2 strip = s*relu(s*Agg(T1) + b1).
    Launch 3 (layer=2): T3 strip = s*((relu((s*Agg(T2)) @ W2 + b2)) @ W3).
    """
    import concourse.mybir as mybir
    import concourse.tile as tile
    from concourse.masks import make_identity

    T = meta["T"]
    f16 = mybir.dt.float16
    f32 = mybir.dt.float32
    Alu = mybir.AluOpType
    nc = _mk_bass()
    dr = _layer_io(nc, meta, layer)
    OUTS_d = nc.dram_tensor("OUTS", [P, T * HID], f16, kind="ExternalOutput")

    with tile.TileContext(nc, num_cores=C) as tc:
        with (
            tc.tile_pool(name="const", bufs=1) as const,
            tc.tile_pool(name="msg", bufs=2) as msgp,
            tc.tile_pool(name="sel", bufs=2) as selp,
            tc.tile_pool(name="zt", bufs=3) as ztp,
            tc.tile_pool(name="psum", bufs=2, space="PSUM") as psp,
        ):
            s = _load_layer_consts(nc, const, dr, meta, layer)
            agg = const.tile([P, T * HID], f16)
            out_strip = const.tile([P, T * HID], f16)
            if layer == 2:
                ident = const.tile([P, P], f16)
                make_identity(nc, ident[:])
                h2_strip = const.tile([P, T * 2 * HID], f16)

            def t3(strip, F=HID):
                return strip[:].rearrange("p (t f) -> p t f", f=F)

            def bdinv(F=HID):
                return s["DINV"][:, :, None].to_broadcast([P, T, F])

            def bbias(F):
                return s["B"][:, None, :].to_broadcast([P, T, F])

            _gather_select(nc, tc, meta, dr["TBL"], s["IDX"], s["MLO"],
                           agg, msgp, selp)
            # u = (agg + self) * dinv
            nc.vector.tensor_tensor(out=agg[:], in0=agg[:], in1=s["SELF"][:],
                                    op=Alu.add)
            nc.vector.tensor_tensor(out=t3(agg), in0=t3(agg), in1=bdinv(),
                                    op=Alu.mult)
            if layer == 1:
                # h1 = relu(u + b1); out = h1 * dinv
                nc.vector.tensor_tensor(out=t3(agg), in0=t3(agg),
                                        in1=bbias(HID), op=Alu.add)
                nc.vector.tensor_scalar(out=agg[:], in0=agg[:], scalar1=0.0,
                                        scalar2=None, op0=Alu.max)
                nc.vector.tensor_tensor(out=t3(out_strip), in0=t3(agg),
                                        in1=bdinv(), op=Alu.mult)
            else:
                # h2 = relu(u @ W2 + b2); out = (h2 @ W3) * dinv
                for t in range(T):
                    psT = psp.tile([HID, P], f16, tag="psT")
                    nc.tensor.transpose(
                        psT[:], agg[:, t * HID:(t + 1) * HID], ident[:])
                    zT = ztp.tile([HID, P], f16, tag="zT")
                    nc.scalar.copy(zT[:], psT[:])
                    ps2 = psp.tile([P, 2 * HID], f32, tag="ps2")
                    nc.tensor.matmul(ps2[:], lhsT=zT[:], rhs=s["W2"][:],
                                     start=True, stop=True)
                    nc.vector.tensor_copy(
                        h2_strip[:, t * 2 * HID:(t + 1) * 2 * HID], ps2[:])
                nc.vector.tensor_tensor(
                    out=t3(h2_strip, 2 * HID), in0=t3(h2_strip, 2 * HID),
                    in1=bbias(2 * HID), op=Alu.add)
                nc.vector.tensor_scalar(out=h2_strip[:], in0=h2_strip[:],
                                        scalar1=0.0, scalar2=None,
                                        op0=Alu.max)
                for t in range(T):
                    psT2 = psp.tile([P, P], f16, tag="psT2")
                    nc.tensor.transpose(
                        psT2[:], h2_strip[:, t * 2 * HID:(t + 1) * 2 * HID],
                        ident[:])
                    hT = ztp.tile([P, P], f16, tag="hT")
                    nc.scalar.copy(hT[:], psT2[:])
                    ps3 = psp.tile([P, HID], f32, tag="ps3")
                    nc.tensor.matmul(ps3[:], lhsT=hT[:], rhs=s["W3"][:],
                                     start=True, stop=True)
                    nc.vector.tensor_copy(
                        out_strip[:, t * HID:(t + 1) * HID], ps3[:])
                nc.vector.tensor_tensor(out=t3(out_strip), in0=t3(out_strip),
                                        in1=bdinv(), op=Alu.mult)
            nc.sync.dma_start(OUTS_d[:], out_strip[:])
    nc.compile()
    return nc


def _prog_final(meta):
    """Launch 4: out3 = s*Agg(T3) + self + b3, then global max pool."""
    import concourse.mybir as mybir
    import concourse.tile as tile
    from concourse.masks import make_identity

    T, Dp = meta["T"], meta["Dp"]
    f16 = mybir.dt.float16
    f32 = mybir.dt.float32
    i16 = mybir.dt.int16
    Alu = mybir.AluOpType
    Axis = mybir.AxisListType
    nc = _mk_bass()
    dr = _layer_io(nc, meta, 3)
    PIDX_d = nc.dram_tensor("PIDX", [P, 8 * Dp], i16, kind="ExternalInput")
    OUT_d = nc.dram_tensor("OUT", [HID, GPC], f32, kind="ExternalOutput")
    out3_local = nc.dram_tensor("out3_local", [1 + P * T, HID], f32)

    with tile.TileContext(nc, num_cores=C) as tc:
        with (
            tc.tile_pool(name="const", bufs=1) as const,
            tc.tile_pool(name="msg", bufs=2) as msgp,
            tc.tile_pool(name="sel", bufs=2) as selp,
            tc.tile_pool(name="zt", bufs=3) as ztp,
            tc.tile_pool(name="psum", bufs=4, space="PSUM") as psp,
        ):
            s = _load_layer_consts(nc, const, dr, meta, 3)
            PIDX_s = const.tile([P, 8 * Dp], i16)
            nc.sync.dma_start(PIDX_s[:], PIDX_d[:])
            ident = const.tile([P, P], f32)
            make_identity(nc, ident[:])
            nirow = const.tile([1, HID], f32)
            nc.vector.memset(nirow[:], float("-inf"))
            nc.sync.dma_start(out3_local[0:1, :], nirow[:])
            agg = const.tile([P, T * HID], f16)
            out3 = const.tile([P, T * HID], f32)

            def t3(strip, F=HID):
                return strip[:].rearrange("p (t f) -> p t f", f=F)

            _gather_select(nc, tc, meta, dr["TBL"], s["IDX"], s["MLO"],
                           agg, msgp, selp)
            nc.vector.tensor_tensor(out=agg[:], in0=agg[:], in1=s["SELF"][:],
                                    op=Alu.add)
            nc.vector.tensor_tensor(
                out=t3(agg), in0=t3(agg),
                in1=s["DINV"][:, :, None].to_broadcast([P, T, HID]),
                op=Alu.mult)
            nc.vector.tensor_tensor(
                out=t3(out3), in0=t3(agg),
                in1=s["B"][:, None, :].to_broadcast([P, T, HID]),
                op=Alu.add)
            nc.sync.dma_start(
                out3_local[1:, :].rearrange("(p t) f -> p (t f)", p=P),
                out3[:])

            pmsg = msgp.tile([P, Dp * HID], f32, tag="pmsg")
            nc.gpsimd.dma_gather(
                out_ap=pmsg[:].rearrange("p (d f) -> p d f", f=HID),
                in_ap=out3_local[:, :],
                idxs_ap=PIDX_s[:],
                num_idxs=P * Dp,
                num_idxs_reg=P * Dp,
                elem_size=HID,
                single_packet=False,
            )
            poolA = ztp.tile([P, HID], f32, tag="poolA")
            nc.vector.tensor_reduce(
                out=poolA[:],
                in_=pmsg[:].rearrange("p (d f) -> p f d", f=HID),
                axis=Axis.X,
                op=Alu.max,
            )
            psP = psp.tile([HID, P], f32, tag="psP")
            nc.tensor.transpose(psP[:], poolA[:], ident[:])
            poolT = ztp.tile([HID, P], f32, tag="poolT")
            nc.vector.tensor_copy(poolT[:], psP[:])
            outsb = ztp.tile([HID, GPC], f32, tag="outsb")
            pt = poolT[:].rearrange("p (g two) -> p g two", two=2)
            nc.vector.tensor_tensor(out=outsb[:], in0=pt[:, :, 0],
                                    in1=pt[:, :, 1], op=Alu.max)
            nc.sync.dma_start(OUT_d[:], outsb[:])
    nc.compile()
    return nc


# --------------------------------------------------------------------------
# Entry point
# --------------------------------------------------------------------------

_RUN_KWARGS = {}
_EXEC_NS = []    # per-launch modeled ns when tracing enabled
_PROFILE = False


def _strips_to_pairs(strips, T, NPAIR):
    """[C][P, T*HID] fp16 strips -> pair table [NPAIR+1, 2*HID] fp16."""
    tab = np.zeros((NPAIR + 1, 2 * HID), np.float16)
    rows = np.concatenate([
        s.reshape(P, T, HID).transpose(1, 0, 2).reshape(T * P, HID)
        for s in strips
    ])                                   # [NT, HID] in global-rank order
    tab[1:] = rows.reshape(NPAIR, 2 * HID)
    return tab


def kernel(data, edge_index, batch, W1, b1, W2, b2, W3, b3):
    from concourse.bass_utils import run_bass_kernel_spmd

    data = np.asarray(data, dtype=np.float32)
    edge_index = np.asarray(edge_index, dtype=np.int32)
    batch_np = np.asarray(batch, dtype=np.int32)
    W1_16 = np.asarray(W1, dtype=np.float16)
    W2_16 = np.asarray(W2, dtype=np.float16)
    W3_16 = np.asarray(W3, dtype=np.float16)
    B1 = np.broadcast_to(np.asarray(b1, np.float16), (P, HID)).copy()
    B2 = np.broadcast_to(np.asarray(b2, np.float16), (P, 2 * HID)).copy()
    B3 = np.broadcast_to(np.asarray(b3, np.float16), (P, HID)).copy()

    prep = _host_prep(data, edge_index, batch_np)
    meta = prep["meta"]
    T, NPAIR = meta["T"], meta["NPAIR"]
    cores = list(range(C))
    del _EXEC_NS[:]

    def run(nc, in_maps):
        if _PROFILE:
            from concourse.timeline_sim import TimelineSim
            _EXEC_NS.append(TimelineSim(nc, require_finite=False).simulate())
        res = run_bass_kernel_spmd(nc, in_maps, cores, **_RUN_KWARGS)
        if res.exec_time_ns is not None:
            _EXEC_NS.append(res.exec_time_ns)
        return res.results

    # ---- launch 1: T1 strips ----
    nc1 = _prog_tables(meta)
    r1 = run(nc1, [{"XT": np.ascontiguousarray(prep["XT"][c]),
                    "W1": W1_16,
                    "DINV": np.ascontiguousarray(prep["dinvT"][c])}
                   for c in range(C)])
    strips = [np.asarray(r1[c]["OUTS"]) for c in range(C)]

    def layer_inputs(c, tab, extra):
        d = {"TBL": tab,
             "SELF": strips[c],
             "DINV": np.ascontiguousarray(prep["dinvT"][c]),
             "IDX16": np.ascontiguousarray(prep["idx16"][c]),
             "MLO": np.ascontiguousarray(prep["mlo"][c])}
        d.update(extra)
        return d

    # ---- launch 2: layer 1 -> T2 strips ----
    tab = _strips_to_pairs(strips, T, NPAIR)
    nc2 = _prog_layer(meta, 1)
    r2 = run(nc2, [layer_inputs(c, tab, {"B": B1}) for c in range(C)])
    strips = [np.asarray(r2[c]["OUTS"]) for c in range(C)]

    # ---- launch 3: layer 2 -> T3 strips ----
    tab = _strips_to_pairs(strips, T, NPAIR)
    nc3 = _prog_layer(meta, 2)
    r3 = run(nc3, [layer_inputs(c, tab, {"B": B2, "W2": W2_16,
                                         "W3": W3_16}) for c in range(C)])
    strips = [np.asarray(r3[c]["OUTS"]) for c in range(C)]

    # ---- launch 4: layer 3 + pool ----
    tab = _strips_to_pairs(strips, T, NPAIR)
    nc4 = _prog_final(meta)
    r4 = run(nc4, [layer_inputs(c, tab,
                                {"B": B3,
                                 "PIDX": np.ascontiguousarray(
                                     prep["pool16"][c])})
                   for c in range(C)])
    out = np.concatenate(
        [np.asarray(r4[c]["OUT"]).T for c in range(C)], axis=0
    )
    return out.astype(np.float32)


# revision 15
# speedup vs baseline: 1.7215x; 1.0861x over previous
"""Trainium2 Bass kernel for a 3-layer GCN encoder with global max pool.

Strategy (8 NeuronCores, SPMD, 4 launches):
  - Nodes are partitioned graph-wise across cores (graphs g -> core g//64),
    degree-sorted within each core so padded-CSR tiles are nearly exact
    (~1.10x padding).
  - GCN normalization is factored: out = s * Agg(s * h), s = 1/sqrt(deg);
    the self loop is a separate "self" add of the core's own table strip.
  - The inter-layer node table is fp16 with TWO nodes per 256-byte row
    ("pair table", 25601 rows), so one int16 index space covers all 51200
    table rows with no low/high split.  Each padded-CSR slot gathers one
    256B pair row; a static uint8 mask selects the needed half:
    ACT copies the hi half, DVE copy_predicated overwrites with the lo
    half where the mask is set (pad slots: row 0 + mask 0 -> zeros).
  - Slots are laid out tile-major with EXACT per-tile depth D_t (max node
    degree within the 128-node tile across all 8 cores); gathers are
    chunked (~6 calls) for DMA/DVE pipelining and reduces run per
    uniform-D run of tiles.
  - Matmuls are reordered to keep every gather 64 features wide:
      L1: T1 = s*(X @ W1);  L2: h1 = relu(s*Agg(T1) + b1), T2 = s*h1
      L3: u = s*Agg(T2); h2 = relu(u @ W2 + b2); T3 = s*(h2 @ W3)
      L4: out3 = s*Agg(T3) + b3; per-graph max pool.
  - The host concatenates per-core strip outputs into the next pair table
    between launches (the "AllGather" costs no device time).
"""

import numpy as np

N_NODES = 50000
N_EDGES = 600000
IN_DIM = 128
HID = 64
N_GRAPHS = 512
C = 8           # cores
P = 128         # partitions
GPC = N_GRAPHS // C


def _pack_idx16(flat):
    """[num] int array -> wrapped [128, num//16] int16 (16-wrapped, 8x repl)."""
    num = flat.shape[0]
    assert num % 16 == 0
    arr = flat.reshape(num // 16, 16).T.astype(np.int16)   # [16, num//16]
    return np.tile(arr, (8, 1))                            # [128, num//16]


# --------------------------------------------------------------------------
# Host-side preprocessing: sharding, permutations, padded CSR index arrays.
# --------------------------------------------------------------------------

def _host_prep(data, edge_index, batch):
    N = data.shape[0]
    src = np.asarray(edge_index[0], dtype=np.int64)
    dst = np.asarray(edge_index[1], dtype=np.int64)
    batch = np.asarray(batch, dtype=np.int64)

    indeg = np.bincount(dst, minlength=N)
    s = (1.0 / np.sqrt((indeg + 1).astype(np.float32))).astype(np.float32)
    core_of_node = batch // GPC

    # degree-sorted rank within each core; rank r -> tile r//P, partition r%P
    perms = []
    for c in range(C):
        nodes_c = np.nonzero(core_of_node == c)[0]
        order = np.argsort(indeg[nodes_c], kind="stable")
        perms.append(nodes_c[order])
    maxlen = max(len(p) for p in perms)
    T = -(-maxlen // P)
    Npc = T * P
    NT = C * Npc
    NPAIR = NT // 2                     # pair rows (table row 0 is the guard)

    tab_of_node = np.full(N, -1, np.int64)
    for c in range(C):
        tab_of_node[perms[c]] = c * Npc + np.arange(len(perms[c]))

    # exact per-tile depth D_t = max indeg among nodes of tile t (all cores)
    deg_of_rank = np.zeros((C, Npc), np.int64)
    for c in range(C):
        deg_of_rank[c, : len(perms[c])] = indeg[perms[c]]
    D_t = deg_of_rank.reshape(C, T, P).max(axis=2).max(axis=0)   # [T]
    slot_off = np.concatenate([[0], np.cumsum(D_t)])             # [T+1]
    S = int(slot_off[-1])

    # edge -> (core, tile, part, slot-within-node)
    vrank = tab_of_node[dst]
    vcore = vrank // Npc
    vloc = vrank % Npc
    vtile = vloc // P
    vpart = vloc % P
    order = np.lexsort((vpart, vtile, vcore))
    ec, et, ep = vcore[order], vtile[order], vpart[order]
    es = src[order]
    key = (ec * T + et) * P + ep
    newgrp = np.concatenate([[True], key[1:] != key[:-1]])
    gsp = np.nonzero(newgrp)[0]
    eslot = np.arange(len(key)) - gsp[np.cumsum(newgrp) - 1]

    # per-core idx (pair row) + lo-half mask arrays, tile-major slot layout
    srow = tab_of_node[es]              # global node rank of the source
    pos = (slot_off[et] + eslot) * P + ep
    idx_flat = np.zeros((C, P * S), np.int64)       # pad -> pair row 0
    mlo_flat = np.zeros((C, P * S), np.uint8)       # pad -> keep hi (zeros)
    for c in range(C):
        m = ec == c
        idx_flat[c][pos[m]] = 1 + srow[m] // 2
        mlo_flat[c][pos[m]] = (srow[m] % 2 == 0).astype(np.uint8)

    # chunks of whole tiles for gather pipelining.  num_idxs per dma_gather
    # must stay below ~16K (SWDGE ring limit) -> cap slots at 115/chunk;
    # later chunks shrink so the post-gather compute tail is short.
    caps = [115, 115, 115, 110, 100, 85, 70, 55, 45, 115, 115]
    chunks = []          # (t0, t1, slot0, nslots)
    t0 = 0
    t = 0
    while t < T:
        cap = caps[min(len(chunks), len(caps) - 1)]
        cn = 0
        t0 = t
        while t < T and (cn == 0 or cn + D_t[t] <= cap):
            cn += int(D_t[t])
            t += 1
        chunks.append((t0, t, int(slot_off[t0]), cn))
    # uniform-D runs within each chunk: (t0, ntiles, D, slot0)
    runs = []
    for (ct0, ct1, cs0, _) in chunks:
        rr = []
        t = ct0
        while t < ct1:
            t2 = t
            while t2 < ct1 and D_t[t2] == D_t[t]:
                t2 += 1
            rr.append((t, t2 - t, int(D_t[t]), int(slot_off[t])))
            t = t2
        runs.append(rr)

    # packed int16 gather indices, chunk-major
    idx16 = []
    for c in range(C):
        cols = []
        for (ct0, ct1, cs0, cn) in chunks:
            if cn:
                cols.append(_pack_idx16(
                    idx_flat[c][cs0 * P: (cs0 + cn) * P]))
        idx16.append(np.concatenate(cols, axis=1) if cols
                     else np.zeros((P, 0), np.int16))
    idx16 = np.stack(idx16)                          # [C, 128, 8*S]
    COLS16 = idx16.shape[2]
    mlo = np.stack([mlo_flat[c].reshape(S, P).T for c in range(C)])  # [C,P,S]

    # dinv per (core, partition, tile)
    dinvT = np.zeros((C, P, T), np.float16)
    for c in range(C):
        n = len(perms[c])
        dv = np.zeros(Npc, np.float32)
        dv[:n] = s[perms[c]]
        dinvT[c] = dv.reshape(T, P).T.astype(np.float16)

    # X^T shard per core (rank-ordered columns), fp16
    XT = np.zeros((C, IN_DIM, Npc), np.float16)
    X = np.asarray(data, dtype=np.float32)
    for c in range(C):
        XT[c, :, : len(perms[c])] = X[perms[c]].T.astype(np.float16)

    # pooling CSR: graph-local g -> partitions 2g, 2g+1 (alternating slots);
    # out3_local row of node (c,t,p) is 1 + p*T + t (row 0 = -inf guard)
    gl = batch % GPC
    loc = tab_of_node % Npc
    tt = loc // P
    pp = loc % P
    okey = core_of_node * (GPC * Npc) + gl * Npc + loc
    oorder = np.argsort(okey)
    oc = core_of_node[oorder]
    ogl = gl[oorder]
    orow = 1 + pp[oorder] * T + tt[oorder]
    gkey = oc * GPC + ogl
    gnew = np.concatenate([[True], gkey[1:] != gkey[:-1]])
    gsp2 = np.nonzero(gnew)[0]
    gslot = np.arange(len(gkey)) - gsp2[np.cumsum(gnew) - 1]
    ppart = 2 * ogl + (gslot % 2)
    pslot = gslot // 2
    Dp = int(pslot.max()) + 1
    pool_flat = np.zeros((C, P * Dp), np.int64)      # pad -> row 0 (-inf)
    pool_flat[oc, pslot * P + ppart] = orow
    pool16 = np.stack([_pack_idx16(pool_flat[c]) for c in range(C)])

    meta = dict(T=T, Npc=Npc, S=S, NPAIR=NPAIR, COLS16=COLS16,
                chunks=chunks, runs=runs, Dp=Dp)
    return dict(idx16=idx16, mlo=mlo, pool16=pool16, dinvT=dinvT, XT=XT,
                meta=meta)


# --------------------------------------------------------------------------
# Bass programs (4 launches)
# --------------------------------------------------------------------------

def _mk_bass():
    import concourse.bacc as bacc
    return bacc.Bacc(None)


def _chunk_gather(nc, meta, ci, TBL_d, idx_tile, MLO_s, msgp, selp):
    """One chunk: dma_gather pair rows + half-select; returns sel tile."""
    import concourse.mybir as mybir
    f16 = mybir.dt.float16
    NPAIR = meta["NPAIR"]
    (ct0, ct1, cs0, cn) = meta["chunks"][ci]
    msg = msgp.tile([P, cn * 2 * HID], f16, tag="msg")
    nc.gpsimd.dma_gather(
        out_ap=msg[:, 0:cn * 2 * HID].rearrange("p (d f) -> p d f", f=2 * HID),
        in_ap=TBL_d[0:NPAIR + 1, :],
        idxs_ap=idx_tile[:, 0:8 * cn],
        num_idxs=P * cn,
        num_idxs_reg=P * cn,
        elem_size=2 * HID,
        single_packet=False,
    )
    sel = selp.tile([P, cn * HID], f16, tag="sel")
    msgv = msg[:, 0:cn * 2 * HID].rearrange("p (d f) -> p d f", f=2 * HID)
    selv = sel[:, 0:cn * HID].rearrange("p (d f) -> p d f", f=HID)
    nc.scalar.copy(selv, msgv[:, :, HID:2 * HID])
    nc.vector.copy_predicated(
        selv,
        MLO_s[:, cs0:cs0 + cn][:, :, None].to_broadcast([P, cn, HID]),
        msgv[:, :, 0:HID],
    )
    return sel


def _chunk_reduce(nc, meta, ci, sel, agg_strip):
    import concourse.mybir as mybir
    Alu = mybir.AluOpType
    Axis = mybir.AxisListType
    (ct0, ct1, cs0, cn) = meta["chunks"][ci]
    with nc.allow_low_precision(reason="fp16 GCN aggregation"):
        for (t0, nt, D, s0) in meta["runs"][ci]:
            sl = agg_strip[:, t0 * HID:(t0 + nt) * HID]
            if D == 0:
                nc.vector.memset(sl, 0.0)
                continue
            lo = s0 - cs0
            nc.vector.tensor_reduce(
                out=sl.rearrange("p (t f) -> p t f", f=HID),
                in_=sel[:, lo * HID:(lo + nt * D) * HID]
                    .rearrange("p (t d f) -> p t f d", f=HID, d=D),
                axis=Axis.X,
                op=Alu.add,
            )


def _prog_tables(meta):
    """Launch 1: T1 strip = s * (X @ W1) for this core's nodes."""
    import concourse.mybir as mybir
    import concourse.tile as tile

    T, Npc = meta["T"], meta["Npc"]
    f16 = mybir.dt.float16
    f32 = mybir.dt.float32
    Alu = mybir.AluOpType
    nc = _mk_bass()

    XT_d = nc.dram_tensor("XT", [IN_DIM, Npc], f16, kind="ExternalInput")
    W1_d = nc.dram_tensor("W1", [IN_DIM, HID], f16, kind="ExternalInput")
    DINV_d = nc.dram_tensor("DINV", [P, T], f16, kind="ExternalInput")
    OUTS_d = nc.dram_tensor("OUTS", [P, T * HID], f16, kind="ExternalOutput")

    with tile.TileContext(nc, num_cores=C) as tc:
        with (
            tc.tile_pool(name="const", bufs=1) as const,
            tc.tile_pool(name="psum", bufs=4, space="PSUM") as psp,
        ):
            XT_s = const.tile([IN_DIM, Npc], f16)
            nc.sync.dma_start(XT_s[:], XT_d[:])
            W1_s = const.tile([IN_DIM, HID], f16)
            nc.sync.dma_start(W1_s[:], W1_d[:])
            DINV_s = const.tile([P, T], f16)
            nc.sync.dma_start(DINV_s[:], DINV_d[:])
            strip = const.tile([P, T * HID], f16)
            for t in range(T):
                ps = psp.tile([P, HID], f32, tag="ps")
                nc.tensor.matmul(ps[:], lhsT=XT_s[:, t * P:(t + 1) * P],
                                 rhs=W1_s[:], start=True, stop=True)
                nc.vector.tensor_copy(strip[:, t * HID:(t + 1) * HID], ps[:])
            nc.vector.tensor_tensor(
                out=strip[:].rearrange("p (t f) -> p t f", f=HID),
                in0=strip[:].rearrange("p (t f) -> p t f", f=HID),
                in1=DINV_s[:, :, None].to_broadcast([P, T, HID]),
                op=Alu.mult,
            )
            nc.sync.dma_start(OUTS_d[:], strip[:])
    nc.compile()
    return nc


def _layer_io(nc, meta, layer):
    import concourse.mybir as mybir
    T = meta["T"]
    f16 = mybir.dt.float16
    i16 = mybir.dt.int16
    u8 = mybir.dt.uint8
    NPAIR = meta["NPAIR"]
    d = {}
    d["TBL"] = nc.dram_tensor("TBL", [NPAIR + 1, 2 * HID], f16,
                              kind="ExternalInput")
    d["SELF"] = nc.dram_tensor("SELF", [P, T * HID], f16,
                               kind="ExternalInput")
    d["DINV"] = nc.dram_tensor("DINV", [P, T], f16, kind="ExternalInput")
    d["IDX16"] = nc.dram_tensor("IDX16", [P, meta["COLS16"]], i16,
                                kind="ExternalInput")
    d["MLO"] = nc.dram_tensor("MLO", [P, meta["S"]], u8,
                              kind="ExternalInput")
    d["B"] = nc.dram_tensor("B", [P, HID if layer != 2 else 2 * HID], f16,
                            kind="ExternalInput")
    if layer == 2:
        d["W2"] = nc.dram_tensor("W2", [HID, 2 * HID], f16,
                                 kind="ExternalInput")
        d["W3"] = nc.dram_tensor("W3", [2 * HID, HID], f16,
                                 kind="ExternalInput")
    return d


def _load_layer_consts(nc, const, dr, meta, layer):
    import concourse.mybir as mybir
    T = meta["T"]
    f16 = mybir.dt.float16
    i16 = mybir.dt.int16
    u8 = mybir.dt.uint8
    s = {}
    s["IDX"] = const.tile([P, meta["COLS16"]], i16, name="IDXs")
    nc.sync.dma_start(s["IDX"][:], dr["IDX16"][:])
    s["MLO"] = const.tile([P, meta["S"]], u8, name="MLOs")
    nc.sync.dma_start(s["MLO"][:], dr["MLO"][:])
    s["SELF"] = const.tile([P, T * HID], f16, name="SELFs")
    nc.sync.dma_start(s["SELF"][:], dr["SELF"][:])
    s["DINV"] = const.tile([P, T], f16, name="DINVs")
    nc.sync.dma_start(s["DINV"][:], dr["DINV"][:])
    s["B"] = const.tile([P, HID if layer != 2 else 2 * HID], f16, name="Bs")
    nc.sync.dma_start(s["B"][:], dr["B"][:])
    if layer == 2:
        s["W2"] = const.tile([HID, 2 * HID], f16, name="W2s")
        nc.sync.dma_start(s["W2"][:], dr["W2"][:])
        s["W3"] = const.tile([2 * HID, HID], f16, name="W3s")
        nc.sync.dma_start(s["W3"][:], dr["W3"][:])
    return s


def _prog_layer(meta, layer):
    """Launch 2 (layer=1): T2 strip = s*relu(s*Agg(T1) + b1).
    Launch 3 (layer=2): T3 strip = s*((relu((s*Agg(T2)) @ W2 + b2)) @ W3).
    Fully chunk-pipelined: chunk k's select/reduce/strip/matmul work
    overlaps chunk k+1's gather DMA.
    """
    import concourse.mybir as mybir
    import concourse.tile as tile
    from concourse.masks import make_identity

    T = meta["T"]
    f16 = mybir.dt.float16
    f32 = mybir.dt.float32
    Alu = mybir.AluOpType
    nc = _mk_bass()
    dr = _layer_io(nc, meta, layer)
    OUTS_d = nc.dram_tensor("OUTS", [P, T * HID], f16, kind="ExternalOutput")

    with tile.TileContext(nc, num_cores=C) as tc:
        with (
            tc.tile_pool(name="const", bufs=1) as const,
            tc.tile_pool(name="idxp", bufs=2) as idxp,
            tc.tile_pool(name="msg", bufs=2) as msgp,
            tc.tile_pool(name="sel", bufs=2) as selp,
            tc.tile_pool(name="zt", bufs=3) as ztp,
            tc.tile_pool(name="psum", bufs=2, space="PSUM") as psp,
        ):
            s = _load_layer_consts(nc, const, dr, meta, layer)
            agg = const.tile([P, T * HID], f16)
            out_strip = const.tile([P, T * HID], f16)
            if layer == 2:
                ident = const.tile([P, P], f16)
                make_identity(nc, ident[:])

            def bd(t0, t1, F=HID):
                return s["DINV"][:, t0:t1, None].to_broadcast([P, t1 - t0, F])

            def v3(ap, F=HID):
                return ap.rearrange("p (t f) -> p t f", f=F)

            for ci, (ct0, ct1, cs0, cn) in enumerate(meta["chunks"]):
                sel = _chunk_gather(nc, meta, ci, dr["TBL"],
                                    s["IDX"][:, 8 * cs0:8 * (cs0 + cn)],
                                    s["MLO"], msgp, selp)
                _chunk_reduce(nc, meta, ci, sel, agg)
                a = agg[:, ct0 * HID:ct1 * HID]
                o = out_strip[:, ct0 * HID:ct1 * HID]
                # u = (agg + self) * dinv
                nc.vector.tensor_tensor(
                    out=a, in0=a, in1=s["SELF"][:, ct0 * HID:ct1 * HID],
                    op=Alu.add)
                nc.vector.tensor_tensor(
                    out=v3(a), in0=v3(a), in1=bd(ct0, ct1), op=Alu.mult)
                if layer == 1:
                    # h1 = relu(u + b1); out = h1 * dinv
                    nc.vector.tensor_tensor(
                        out=v3(a), in0=v3(a),
                        in1=s["B"][:, None, :].to_broadcast(
                            [P, ct1 - ct0, HID]),
                        op=Alu.add)
                    nc.vector.tensor_scalar(out=a, in0=a, scalar1=0.0,
                                            scalar2=None, op0=Alu.max)
                    nc.vector.tensor_tensor(
                        out=v3(o), in0=v3(a), in1=bd(ct0, ct1), op=Alu.mult)
                else:
                    # per tile: h2 = relu(u @ W2 + b2); out = (h2 @ W3)*dinv
                    for t in range(ct0, ct1):
                        ut = agg[:, t * HID:(t + 1) * HID]
                        psT = psp.tile([HID, P], f16, tag="psT")
                        nc.tensor.transpose(psT[:], ut, ident[:])
                        zT = ztp.tile([HID, P], f16, tag="zT")
                        nc.scalar.copy(zT[:], psT[:])
                        ps2 = psp.tile([P, 2 * HID], f32, tag="ps2")
                        nc.tensor.matmul(ps2[:], lhsT=zT[:], rhs=s["W2"][:],
                                         start=True, stop=True)
                        h2t = ztp.tile([P, 2 * HID], f16, tag="h2t")
                        nc.vector.tensor_tensor(out=h2t[:], in0=ps2[:],
                                                in1=s["B"][:], op=Alu.add)
                        nc.vector.tensor_scalar(out=h2t[:], in0=h2t[:],
                                                scalar1=0.0, scalar2=None,
                                                op0=Alu.max)
                        psT2 = psp.tile([P, P], f16, tag="psT2")
                        nc.tensor.transpose(psT2[:], h2t[:], ident[:])
                        hT = ztp.tile([P, P], f16, tag="hT")
                        nc.scalar.copy(hT[:], psT2[:])
                        ps3 = psp.tile([P, HID], f32, tag="ps3")
                        nc.tensor.matmul(ps3[:], lhsT=hT[:], rhs=s["W3"][:],
                                         start=True, stop=True)
                        nc.vector.tensor_tensor(
                            out=v3(out_strip[:, t * HID:(t + 1) * HID]),
                            in0=ps3[:, None, :],
                            in1=bd(t, t + 1), op=Alu.mult)
                nc.sync.dma_start(OUTS_d[:, ct0 * HID:ct1 * HID], o)
    nc.compile()
    return nc


def _prog_final(meta):
    """Launch 4: out3 = s*Agg(T3) + self + b3, then global max pool."""
    import concourse.mybir as mybir
    import concourse.tile as tile
    from concourse.masks import make_identity

    T, Dp = meta["T"], meta["Dp"]
    f16 = mybir.dt.float16
    f32 = mybir.dt.float32
    i16 = mybir.dt.int16
    Alu = mybir.AluOpType
    Axis = mybir.AxisListType
    nc = _mk_bass()
    dr = _layer_io(nc, meta, 3)
    PIDX_d = nc.dram_tensor("PIDX", [P, 8 * Dp], i16, kind="ExternalInput")
    OUT_d = nc.dram_tensor("OUT", [HID, GPC], f32, kind="ExternalOutput")
    out3_local = nc.dram_tensor("out3_local", [1 + P * T, HID], f32)
    o3rows = out3_local[1:, :].rearrange("(p t) f -> p t f", p=P)

    with tile.TileContext(nc, num_cores=C) as tc:
        with (
            tc.tile_pool(name="const", bufs=1) as const,
            tc.tile_pool(name="idxp", bufs=2) as idxp,
            tc.tile_pool(name="msg", bufs=2) as msgp,
            tc.tile_pool(name="sel", bufs=2) as selp,
            tc.tile_pool(name="zt", bufs=3) as ztp,
            tc.tile_pool(name="psum", bufs=2, space="PSUM") as psp,
        ):
            s = _load_layer_consts(nc, const, dr, meta, 3)
            PIDX_s = const.tile([P, 8 * Dp], i16)
            nc.sync.dma_start(PIDX_s[:], PIDX_d[:])
            ident = const.tile([P, P], f32)
            make_identity(nc, ident[:])
            nirow = const.tile([1, HID], f32)
            nc.vector.memset(nirow[:], float("-inf"))
            nc.sync.dma_start(out3_local[0:1, :], nirow[:])
            agg = const.tile([P, T * HID], f16)
            out3 = const.tile([P, T * HID], f32)

            def bd(t0, t1, F=HID):
                return s["DINV"][:, t0:t1, None].to_broadcast([P, t1 - t0, F])

            def v3(ap, F=HID):
                return ap.rearrange("p (t f) -> p t f", f=F)

            for ci, (ct0, ct1, cs0, cn) in enumerate(meta["chunks"]):
                idx_tile = idxp.tile([P, 8 * cn], mybir.dt.int16, tag="idx",
                                     name="idxt")
                nc.sync.dma_start(idx_tile[:, 0:8 * cn],
                                  dr["IDX16"][:, 8 * cs0:8 * (cs0 + cn)])
                sel = _chunk_gather(nc, meta, ci, dr["TBL"], idx_tile,
                                    s["MLO"], msgp, selp)
                _chunk_reduce(nc, meta, ci, sel, agg)
                a = agg[:, ct0 * HID:ct1 * HID]
                o = out3[:, ct0 * HID:ct1 * HID]
                nc.vector.tensor_tensor(
                    out=a, in0=a, in1=s["SELF"][:, ct0 * HID:ct1 * HID],
                    op=Alu.add)
                nc.vector.tensor_tensor(
                    out=v3(a), in0=v3(a), in1=bd(ct0, ct1), op=Alu.mult)
                nc.vector.tensor_tensor(
                    out=v3(o), in0=v3(a),
                    in1=s["B"][:, None, :].to_broadcast([P, ct1 - ct0, HID]),
                    op=Alu.add)
                nc.sync.dma_start(o3rows[:, ct0:ct1, :], v3(o))

            pmsg = msgp.tile([P, Dp * HID], f32, tag="pmsg")
            nc.gpsimd.dma_gather(
                out_ap=pmsg[:].rearrange("p (d f) -> p d f", f=HID),
                in_ap=out3_local[:, :],
                idxs_ap=PIDX_s[:],
                num_idxs=P * Dp,
                num_idxs_reg=P * Dp,
                elem_size=HID,
                single_packet=False,
            )
            poolA = ztp.tile([P, HID], f32, tag="poolA")
            nc.vector.tensor_reduce(
                out=poolA[:],
                in_=pmsg[:].rearrange("p (d f) -> p f d", f=HID),
                axis=Axis.X,
                op=Alu.max,
            )
            psP = psp.tile([HID, P], f32, tag="psP")
            nc.tensor.transpose(psP[:], poolA[:], ident[:])
            poolT = ztp.tile([HID, P], f32, tag="poolT")
            nc.vector.tensor_copy(poolT[:], psP[:])
            outsb = ztp.tile([HID, GPC], f32, tag="outsb")
            pt = poolT[:].rearrange("p (g two) -> p g two", two=2)
            nc.vector.tensor_tensor(out=outsb[:], in0=pt[:, :, 0],
                                    in1=pt[:, :, 1], op=Alu.max)
            nc.sync.dma_start(OUT_d[:], outsb[:])
    nc.compile()
    return nc


# --------------------------------------------------------------------------
# Entry point
# --------------------------------------------------------------------------

_RUN_KWARGS = {}
_EXEC_NS = []    # per-launch modeled ns when tracing enabled
_PROFILE = False


def _strips_to_pairs(strips, T, NPAIR):
    """[C][P, T*HID] fp16 strips -> pair table [NPAIR+1, 2*HID] fp16."""
    tab = np.zeros((NPAIR + 1, 2 * HID), np.float16)
    rows = np.concatenate([
        s.reshape(P, T, HID).transpose(1, 0, 2).reshape(T * P, HID)
        for s in strips
    ])                                   # [NT, HID] in global-rank order
    tab[1:] = rows.reshape(NPAIR, 2 * HID)
    return tab


def kernel(data, edge_index, batch, W1, b1, W2, b2, W3, b3):
    from concourse.bass_utils import run_bass_kernel_spmd

    data = np.asarray(data, dtype=np.float32)
    edge_index = np.asarray(edge_index, dtype=np.int32)
    batch_np = np.asarray(batch, dtype=np.int32)
    W1_16 = np.asarray(W1, dtype=np.float16)
    W2_16 = np.asarray(W2, dtype=np.float16)
    W3_16 = np.asarray(W3, dtype=np.float16)
    B1 = np.broadcast_to(np.asarray(b1, np.float16), (P, HID)).copy()
    B2 = np.broadcast_to(np.asarray(b2, np.float16), (P, 2 * HID)).copy()
    B3 = np.broadcast_to(np.asarray(b3, np.float16), (P, HID)).copy()

    prep = _host_prep(data, edge_index, batch_np)
    meta = prep["meta"]
    T, NPAIR = meta["T"], meta["NPAIR"]
    cores = list(range(C))
    del _EXEC_NS[:]

    def run(nc, in_maps):
        if _PROFILE:
            from concourse.timeline_sim import TimelineSim
            _EXEC_NS.append(TimelineSim(nc, require_finite=False).simulate())
        res = run_bass_kernel_spmd(nc, in_maps, cores, **_RUN_KWARGS)
        if res.exec_time_ns is not None:
            _EXEC_NS.append(res.exec_time_ns)
        return res.results

    # ---- launch 1: T1 strips ----
    nc1 = _prog_tables(meta)
    r1 = run(nc1, [{"XT": np.ascontiguousarray(prep["XT"][c]),
                    "W1": W1_16,
                    "DINV": np.ascontiguousarray(prep["dinvT"][c])}
                   for c in range(C)])
    strips = [np.asarray(r1[c]["OUTS"]) for c in range(C)]

    def layer_inputs(c, tab, extra):
        d = {"TBL": tab,
             "SELF": strips[c],
             "DINV": np.ascontiguousarray(prep["dinvT"][c]),
             "IDX16": np.ascontiguousarray(prep["idx16"][c]),
             "MLO": np.ascontiguousarray(prep["mlo"][c])}
        d.update(extra)
        return d

    # ---- launch 2: layer 1 -> T2 strips ----
    tab = _strips_to_pairs(strips, T, NPAIR)
    nc2 = _prog_layer(meta, 1)
    r2 = run(nc2, [layer_inputs(c, tab, {"B": B1}) for c in range(C)])
    strips = [np.asarray(r2[c]["OUTS"]) for c in range(C)]

    # ---- launch 3: layer 2 -> T3 strips ----
    tab = _strips_to_pairs(strips, T, NPAIR)
    nc3 = _prog_layer(meta, 2)
    r3 = run(nc3, [layer_inputs(c, tab, {"B": B2, "W2": W2_16,
                                         "W3": W3_16}) for c in range(C)])
    strips = [np.asarray(r3[c]["OUTS"]) for c in range(C)]

    # ---- launch 4: layer 3 + pool ----
    tab = _strips_to_pairs(strips, T, NPAIR)
    nc4 = _prog_final(meta)
    r4 = run(nc4, [layer_inputs(c, tab,
                                {"B": B3,
                                 "PIDX": np.ascontiguousarray(
                                     prep["pool16"][c])})
                   for c in range(C)])
    out = np.concatenate(
        [np.asarray(r4[c]["OUT"]).T for c in range(C)], axis=0
    )
    return out.astype(np.float32)
